# revision 12
# baseline (speedup 1.0000x reference)
"""DAM-Softmax loss kernel for Trainium2 (Bass/Tile), 8-core data parallel.

Math (per sample b, target t = label[b]):
    cos_t  = costh[b, t]
    delta  = (MARGIN/LAMDA) * exp(1 - cos_t)
    logits = S * costh, with logits[b, t] = S * (cos_t - delta)
    loss_b = logsumexp(logits[b, :]) - S * (cos_t - delta)
    loss   = mean_b loss_b

costh is bounded in [0, 1), so M = 1.0 is the stability shift:
    ssum   = sum_j exp(S*(costh[b,j] - M))
    Z      = ssum - exp(S*(cos_t - M)) + exp(S*(cos_t - delta - M))
    loss_b = S*M + ln(Z) - S*(cos_t - delta)

Performance structure (per core: [1024, 10000] shard, 10.24M exps):

* The bulk ssum term only needs ~1% accuracy (harness tolerance is 2e-2
  on the final scalar; per-element errors average out over 10000
  columns), so the stream tensor is staged in HBM as fp8 e4m3 -- 4x
  fewer bytes than f32 (DMA 10.24 MB/core ~ 28 us at ~360 GB/s/core).
* exp+row-sum at full fp32 spline accuracy runs only on ScalarE (ACT,
  1 elem/cycle/lane @ 1.2 GHz = 66.7 us for the whole shard -- the
  bottleneck).  So the columns are split: ACT handles WA columns with
  fused exp+accumulate, and VectorE (DVE) handles the remaining WD
  columns with a Schraudolph-style exp2:
      pass1: i16 = rint(x * (S*log2e*128) + (127 - S*log2e)*128)
             (one tensor_scalar, f32 internal, int16 round-on-write)
      pass2: bitcast i16 -> bf16 gives 2^z with mantissa-linear
             interpolation error (+0..+6%, mean +4%); a second
             tensor_scalar accumulates it per row (4x perf mode).
  The deterministic quantization/interpolation biases of both paths are
  removed by constant factors computed offline from the rounding rules
  (equidistribution within bins -- input-independent).
* Per-sample target terms (cos_t gather, margin, e1/e2, -S*ct_adj) are
  computed from the untouched f32 input: exact where it matters.

WA/WD ~ 6040/3960 balances ACT (WA cycles @ 1.2 GHz) against DVE
(WD @ 1x for the fp8 pass1 + WD/4 @ 4x for pass2, @ 0.96 GHz): both
~ 42 us, overlapped with the 28 us DMA stream.

Sharding: batch dim split evenly across 8 NeuronCores (data parallel);
host mean-reduces the 8 x [128, 8] per-sample loss outputs.
"""

import numpy as np
import ml_dtypes

NCORES = 8
B, C = 8192, 10000
R = B // NCORES          # rows per core
P = 128                  # SBUF partitions
T = R // P               # row tiles per core
S = 15.0
MARGIN = 0.3
LAMDA = 2.0
DCOEF = MARGIN / LAMDA
MAXC = 1.0               # upper bound of costh (uniform [0,1)) used as exp shift

FP8 = ml_dtypes.float8_e4m3
LOG2E = float(np.log2(np.e))
SCHRA_A = S * LOG2E * 128.0             # pass1: code = rint(x*A + B)
SCHRA_B = (127.0 - S * LOG2E) * 128.0
WD = 3960                # columns handled by DVE (Schraudolph)
WA = C - WD              # columns handled by ACT (true exp)


def _schraudolph_np(x64):
    """Bit-exact numpy model of the DVE pass1+pass2 pipeline."""
    codes = np.rint(x64 * SCHRA_A + SCHRA_B).astype(np.int16)
    return codes.view(ml_dtypes.bfloat16).astype(np.float64)


def _debias():
    """Constant bias factors of the two approximate paths vs true exp,
    for equidistributed in-bin rounding errors (input-independent).

    ACT path: fp8 quantization only.  DVE path: fp8 quantization +
    Schraudolph mantissa-linear interpolation + int16 rounding.
    """
    x = (np.arange(1 << 20, dtype=np.float64) + 0.5) / (1 << 20)
    xq = x.astype(np.float32).astype(FP8).astype(np.float64)
    num = np.exp(S * (x - 1.0)).sum()
    act = num / np.exp(S * (xq - 1.0)).sum()
    dve = num / _schraudolph_np(xq).sum()
    return float(act), float(dve)


DEBIAS_ACT, DEBIAS_DVE = _debias()

_NC_CACHE = {}


def _build_nc(repeat=1, big_bufs=4, loop_reps=1, wd=WD):
    # repeat > 1 re-streams the shard `repeat` times inside one NEFF; used by
    # the timing harness to infer per-pass device time from the wall-clock
    # slope (axon dispatch overhead cancels in the difference).  loop_reps > 1
    # additionally wraps the passes in a hardware For_i loop (amplifies
    # device time without growing the NEFF, at ~10% loop-sync tax).
    import concourse.bacc as bacc
    import concourse.bass as bass
    import concourse.mybir as mybir
    import concourse.tile as tile

    f32 = mybir.dt.float32
    fp8 = mybir.dt.float8e4
    bf16 = mybir.dt.bfloat16
    i16 = mybir.dt.int16
    i32 = mybir.dt.int32
    Act = mybir.ActivationFunctionType
    Alu = mybir.AluOpType

    wa = C - wd
    nc = bacc.Bacc(None, target_bir_lowering=False, debug=False)

    costh = nc.dram_tensor("costh", [R, C], f32, kind="ExternalInput")
    costh8 = nc.dram_tensor("costh8", [R, C], fp8, kind="ExternalInput")
    label = nc.dram_tensor("label", [R], i32, kind="ExternalInput")
    out = nc.dram_tensor("out", [P, T], f32, kind="ExternalOutput")

    with tile.TileContext(nc) as tc:
        with (
            tc.tile_pool(name="big", bufs=big_bufs) as big,
            tc.tile_pool(name="codes", bufs=2) as cpool,
            tc.tile_pool(name="small", bufs=1) as small,
        ):
            # bias vector for exp(S*x - S*M) activations
            neg_sm = small.tile([P, 1], f32)
            nc.vector.memset(neg_sm[:], -S * MAXC)

            # --- prologue: gather target cosines cos_t[p, t] = costh[t*P+p, label] ---
            label_sb = small.tile([P, T], i32)
            nc.gpsimd.dma_start(
                out=label_sb[:], in_=label[:].rearrange("(t p) -> p t", p=P)
            )
            # idx[p, t] = (t*P + p) * C + label  (flat element index), computed
            # in f32 (exact: values < 2^24) since iota steps are limited to i16.
            row_i = small.tile([P, T], i32)
            nc.gpsimd.iota(row_i[:], pattern=[[P, T]], base=0, channel_multiplier=1)
            row_f = small.tile([P, T], f32)
            nc.vector.tensor_copy(out=row_f[:], in_=row_i[:])
            lab_f = small.tile([P, T], f32)
            nc.vector.tensor_copy(out=lab_f[:], in_=label_sb[:])
            idx_f = small.tile([P, T], f32)
            nc.vector.scalar_tensor_tensor(
                out=idx_f[:], in0=row_f[:], scalar=float(C), in1=lab_f[:],
                op0=Alu.mult, op1=Alu.add,
            )
            idx = small.tile([P, T], i32)
            nc.vector.tensor_copy(out=idx[:], in_=idx_f[:])
            # one indirect DMA per column: HW honors only one index per
            # partition per gather (multi-column offset APs misbehave on HW)
            cos_t = small.tile([P, T], f32)
            for t in range(T):
                nc.gpsimd.indirect_dma_start(
                    out=cos_t[:, t:t + 1],
                    out_offset=None,
                    in_=costh[:, :],
                    in_offset=bass.IndirectOffsetOnAxis(ap=idx[:, t:t + 1], axis=1),
                )

            # target-term math depends only on cos_t, so it is emitted before
            # the stream and overlaps it:
            #   delta_e = exp(1 - cos_t);  ct_adj = cos_t - DCOEF * delta_e
            #   e12 = exp(S*(cos_t - M)) - exp(S*(ct_adj - M))
            delta_e = small.tile([P, T], f32)
            nc.scalar.activation(
                out=delta_e[:], in_=cos_t[:], func=Act.Exp, bias=1.0, scale=-1.0
            )
            ct_adj = small.tile([P, T], f32)
            nc.vector.scalar_tensor_tensor(
                out=ct_adj[:], in0=delta_e[:], scalar=-DCOEF, in1=cos_t[:],
                op0=Alu.mult, op1=Alu.add,
            )
            e1 = small.tile([P, T], f32)
            nc.scalar.activation(
                out=e1[:], in_=cos_t[:], func=Act.Exp, bias=neg_sm[:], scale=S
            )
            e2 = small.tile([P, T], f32)
            nc.scalar.activation(
                out=e2[:], in_=ct_adj[:], func=Act.Exp, bias=neg_sm[:], scale=S
            )
            e12 = small.tile([P, T], f32)
            nc.vector.tensor_sub(out=e12[:], in0=e1[:], in1=e2[:])

            # --- main loop: stream fp8 shard; ACT does true exp+accum on
            # cols [0, wa), DVE does Schraudolph exp2 on cols [wa, C) ---
            exp_scr = small.tile([P, wa], f32)     # ACT main out (scratch)
            scr16 = small.tile([P, wd], bf16)      # DVE pass2 main out (scratch)
            sA = small.tile([P, T], f32)
            sD = small.tile([P, T], f32)

            def one_pass():
                for t in range(T):
                    xa = big.tile([P, wa], fp8, tag="xa")
                    nc.sync.dma_start(
                        out=xa[:], in_=costh8[t * P:(t + 1) * P, 0:wa]
                    )
                    nc.scalar.activation(
                        out=exp_scr[:], in_=xa[:], func=Act.Exp,
                        bias=neg_sm[:], scale=S,
                        accum_out=sA[:, t:t + 1],
                    )
                    xd = big.tile([P, wd], fp8, tag="xd")
                    nc.sync.dma_start(
                        out=xd[:], in_=costh8[t * P:(t + 1) * P, wa:C]
                    )
                    codes = cpool.tile([P, wd], i16, tag="codes")
                    nc.vector.tensor_scalar(
                        out=codes[:], in0=xd[:], scalar1=SCHRA_A,
                        scalar2=SCHRA_B, op0=Alu.mult, op1=Alu.add,
                    )
                    nc.vector.tensor_scalar(
                        out=scr16[:], in0=codes[:].bitcast(bf16),
                        scalar1=1.0, scalar2=0.0, op0=Alu.mult, op1=Alu.add,
                        accum_out=sD[:, t:t + 1],
                    )

            if loop_reps > 1:
                with tc.For_i(0, loop_reps, 1):
                    for _rep in range(repeat):
                        one_pass()
            for _rep in range(repeat):
                one_pass()

            # --- tail: z = DEBIAS_ACT*sA + DEBIAS_DVE*sD - e12;
            #     loss_dev = ln(z) - S*ct_adj ---
            zd = small.tile([P, T], f32)
            nc.vector.scalar_tensor_tensor(
                out=zd[:], in0=sD[:], scalar=DEBIAS_DVE, in1=e12[:],
                op0=Alu.mult, op1=Alu.subtract,
            )
            z = small.tile([P, T], f32)
            nc.vector.scalar_tensor_tensor(
                out=z[:], in0=sA[:], scalar=DEBIAS_ACT, in1=zd[:],
                op0=Alu.mult, op1=Alu.add,
            )
            lnz = small.tile([P, T], f32)
            nc.scalar.activation(out=lnz[:], in_=z[:], func=Act.Ln)
            loss = small.tile([P, T], f32)
            nc.vector.scalar_tensor_tensor(
                out=loss[:], in0=ct_adj[:], scalar=-S, in1=lnz[:],
                op0=Alu.mult, op1=Alu.add,
            )
            nc.sync.dma_start(out=out[:], in_=loss[:])

    nc.compile()
    return nc


def _get_nc():
    if "nc" not in _NC_CACHE:
        _NC_CACHE["nc"] = _build_nc()
    return _NC_CACHE["nc"]


def _full_inputs(costh, label):
    """Full (unsharded) input arrays keyed by dram tensor name."""
    costh = np.ascontiguousarray(costh, dtype=np.float32)
    return {
        "costh": costh,
        "costh8": costh.astype(FP8),
        "label": np.ascontiguousarray(label).astype(np.int32),
    }


def _run(costh_np, label_np, trace=False, **spmd_kwargs):
    from concourse.bass_utils import run_bass_kernel_spmd

    nc = _get_nc()
    full = _full_inputs(costh_np, label_np)
    in_maps = [
        {name: arr[k * R:(k + 1) * R] for name, arr in full.items()}
        for k in range(NCORES)
    ]
    # The first execution of a fresh NEFF through the axon tunnel
    # occasionally faults with NRT_EXEC_UNIT_UNRECOVERABLE; failures are
    # loud (exception, never silent corruption), so a bounded retry is safe.
    # A non-finite total also triggers a retry as extra insurance.
    last_exc = None
    for _attempt in range(3):
        try:
            res = run_bass_kernel_spmd(
                nc, in_maps, core_ids=list(range(NCORES)), trace=trace,
                **spmd_kwargs
            )
            total = sum(r["out"].astype(np.float64).sum() for r in res.results)
            if np.isfinite(total):
                break
            last_exc = RuntimeError("non-finite loss from device")
        except Exception as exc:  # noqa: BLE001
            last_exc = exc
    else:
        raise last_exc
    loss = np.float32(total / B + S * MAXC)
    return loss, res


def kernel(costh, label):
    loss, _ = _run(costh, label)
    return loss
